# revision 18
# baseline (speedup 1.0000x reference)
"""CoAttention kernel for Trainium2, data-parallel over batch across 8 NeuronCores.

Per core (one batch element b):
    query = data1[b] @ Wq + bq                      # [2048, 256]
    key   = data2[b] @ Wk + bk                      # [2048, 256]
    attn  = softmax(SCALE * query @ key^T)          # row-constant terms cancel
    out   = attn @ key + query

Device-side strategy (v7):
  - The host uploads d1^T and d2^T in bf16, i-interleaved p-major so
    every DMA descriptor is one 8KB contiguous run per partition, and
    d1^T is split into four q-chunks so each QT projection unit
    completes as its chunk lands (the weight rows are packed in the
    matching i-permutation; contraction order is free).  No device
    casts, no input transposes, ~5.6 MiB/core input traffic.
  - softmax(q@(k+bk)^T) drops bias terms constant along k, and
    sum(attn)==1 makes attn@(key+bk) == attn@key + bk, so the key value
    matrix carries NO bias; bq biases the scores path and (bq+bk) the
    residual path from the same QT PSUM.  The softmax denominator is a
    memset 1.0 column appended to the fp8 key values.
  - Scores-path QT evicts split ACT/DVE so exp #0 fires ~1us after the
    second d1 chunk lands.  key values come from fp8 PE transposes of
    kt (stride-2 PSUM), not a second projection matmul.
  - Residual Q reaches [q, d] layout via xbar DMA transposes on the
    idle mid-kernel DMA engines: no PE, no PSUM, no DVE eviction.
  - scoresT [k, q] orientation lets exp(scoresT) feed the context
    matmul as the stationary operand; scores and context run in fp8e4m3
    DoubleRow.  ctx for the second q-half is split kp0-6 (inside the
    exp stream, evicted to bf16) + kp7 (after the last exp); post-exp
    divides run on the then-idle ACT engine (activation scale=recip AP).
  - Output is written in 8 chunks of 256 rows as each completes.
"""

import sys

if "/opt/trn_rl_repo" not in sys.path:
    sys.path.insert(0, "/opt/trn_rl_repo")

from contextlib import ExitStack

import ml_dtypes
import numpy as np

import concourse.bass as bass  # noqa: F401
import concourse.mybir as mybir
import concourse.tile as tile
from concourse import bacc
from concourse.bass_utils import run_bass_kernel_spmd

B, LQ, LK, DIN, D = 8, 2048, 2048, 1024, 256
N_CORES = 8
SCALE = float(1.0 / np.sqrt(1024.0).astype(np.float32))

BF16 = mybir.dt.bfloat16
FP8 = mybir.dt.float8e4
F32 = mybir.dt.float32
AF = mybir.ActivationFunctionType
PM_DR = mybir.MatmulPerfMode.DoubleRow
ADD = mybir.AluOpType.add
MULT = mybir.AluOpType.mult

QB = 16           # q blocks of 128
KB = 16           # k blocks of 128
J1 = 8            # d1 i-interleave factor (1024 = 128 * 8)
J2 = 2            # d2 i-interleave factor (256 = 128 * 2)
KP = KB // 2      # 8 fp8 DoubleRow k-pairs
KP_A = 7          # h1 ctx kp-split: A = kp0-6 inside exp stream, B = kp7 after


def _build():
    nc = bacc.Bacc("TRN2", target_bir_lowering=False, debug=False)
    d1t = nc.dram_tensor("d1t", [8, 128, J1, 256], BF16, kind="ExternalInput").ap()
    d2t = nc.dram_tensor("d2t", [128, J2, LK], FP8, kind="ExternalInput").ap()
    wq_d = nc.dram_tensor("wq", [128, 2048], BF16, kind="ExternalInput").ap()
    wk_d = nc.dram_tensor("wk", [128, 512], FP8, kind="ExternalInput").ap()
    bias = nc.dram_tensor("bias", [128, 4], F32, kind="ExternalInput").ap()
    out = nc.dram_tensor("out", [LQ, D], F32, kind="ExternalOutput").ap()

    with tile.TileContext(nc) as tc, ExitStack() as ctx:
        const = ctx.enter_context(tc.tile_pool(name="const", bufs=1))
        big = ctx.enter_context(tc.tile_pool(name="big", bufs=1))
        stage = ctx.enter_context(tc.tile_pool(name="stage", bufs=3))
        small = ctx.enter_context(tc.tile_pool(name="small", bufs=4))
        ps_a = ctx.enter_context(tc.tile_pool(name="ps_a", bufs=4, space="PSUM"))
        ps_sc = ctx.enter_context(tc.tile_pool(name="ps_sc", bufs=2, space="PSUM"))

        # ---------------- constants / small state ---------------------------
        warm_src = const.tile([128, 512], BF16, tag="warm_src")
        nc.gpsimd.memset(warm_src[:], 0.0)
        dummy = const.tile([128, 1], F32, tag="dummy")
        # force the exp ACT table load at kernel start (otherwise it stalls
        # the first real exp by ~1.3us mid-stream)
        nc.scalar.activation(dummy[:], warm_src[:, 0:1], AF.Exp)

        key2 = [
            big.tile([128, 2, D + 1], FP8, tag=f"key2_{kp}", name=f"key2_{kp}")
            for kp in range(KP)
        ]
        for kp in range(KP):
            nc.gpsimd.memset(key2[kp][:, :, D:D + 1], 1.0)

        # ---------------- loads ---------------------------------------------
        wq_sb = const.tile([128, 2048], BF16, tag="wq_sb")
        wk_sb = const.tile([128, 512], FP8, tag="wk_sb")
        bias_sb = const.tile([128, 4], F32, tag="bias_sb")
        d2T = big.tile([128, J2, LK], FP8, tag="d2T")
        d1T = [big.tile([128, J1, 256], BF16, tag=f"d1T{n}", name=f"d1T{n}")
               for n in range(8)]

        nc.sync.dma_start(out=wq_sb[:], in_=wq_d)
        for n in range(2):
            nc.sync.dma_start(out=d1T[n][:], in_=d1t[n])
        nc.sync.dma_start(out=bias_sb[:], in_=bias)
        nc.sync.dma_start(out=wk_sb[:], in_=wk_d)
        for n in range(2, 4):
            nc.sync.dma_start(out=d1T[n][:], in_=d1t[n])
        nc.sync.dma_start(out=d2T[:], in_=d2t)
        for n in range(4, 8):
            nc.sync.dma_start(out=d1T[n][:], in_=d1t[n])

        # weight slices in the same i-permutation as the activations
        wqs = [wq_sb[:, j * D:(j + 1) * D] for j in range(J1)]
        wks = [wk_sb[:, j * D:(j + 1) * D] for j in range(J2)]
        bq_col = bias_sb[:, 0:2]
        bqk_col = bias_sb[:, 2:4]

        # ---------------- PE p-state warmup (also bridges the d2T wait) -----
        for w in range(7):
            pw = ps_a.tile([128, 512], F32, tag="ps_a", name=f"warm{w}")
            nc.tensor.matmul(pw[:], lhsT=warm_src[:, :128], rhs=warm_src[:],
                             start=True, stop=True)

        # ---------------- K^T fp8 DoubleRow layout [128, 2, k] --------------
        kt_sb = big.tile([128, 2, LK], FP8, tag="kt_sb")

        wk2 = wk_sb[:].rearrange("p (j d) -> p j d", j=J2)

        def kt_unit(dc, nk, on_act):
            ps = ps_a.tile([128, 512], F32, tag="ps_a")
            nc.tensor.matmul(
                ps[:],
                lhsT=wk2[:, :, dc * 128:(dc + 1) * 128],
                rhs=d2T[:, :, nk * 512:(nk + 1) * 512],
                perf_mode=PM_DR,
                start=True,
                stop=True,
            )
            o = kt_sb[:, dc, nk * 512:(nk + 1) * 512]
            if on_act:
                nc.scalar.copy(o, ps[:])
            else:
                nc.vector.tensor_copy(o, ps[:])

        # ---------------- key values via fp8 DoubleRow matmuls ---------------
        def key_tr(kp):
            ps = ps_a.tile([128, 512], F32, tag="ps_a")
            for s in range(2):
                kb = 2 * kp + s
                nc.tensor.matmul(
                    ps[:, s * D:(s + 1) * D],
                    lhsT=d2T[:, :, kb * 128:(kb + 1) * 128],
                    rhs=wk2,
                    perf_mode=PM_DR,
                    start=True,
                    stop=True,
                )
            nc.vector.tensor_copy(
                key2[kp][:, :, :D],
                ps[:].rearrange("p (s d) -> p s d", s=2),
            )

        # ---------------- QT projection ------------------------------------
        qt_sb = big.tile([128, 2, LQ], FP8, tag="qt_sb")
        qtbf = big.tile([128, 2, LQ], BF16, tag="qtbf")

        def qt_bias_sc(ps, dc, nq, on_act):
            o = qt_sb[:, dc, nq * 512:(nq + 1) * 512]
            if on_act:
                nc.scalar.activation(o, ps[:], AF.Identity,
                                     bias=bq_col[:, dc:dc + 1])
            else:
                nc.vector.tensor_scalar(o, ps[:], bq_col[:, dc:dc + 1], None, ADD)

        def qt_bias_rs(ps, dc, nq, on_act):
            o = qtbf[:, dc, nq * 512:(nq + 1) * 512]
            if on_act:
                nc.scalar.activation(o, ps[:], AF.Identity,
                                     bias=bqk_col[:, dc:dc + 1])
            else:
                nc.vector.tensor_scalar(o, ps[:], bqk_col[:, dc:dc + 1], None, ADD)

        qt_ps = {}

        def qt_half(dc, nq, h):
            # one 8-chain over q-chunk c = 2*nq + h into half of the psum tile
            if h == 0:
                qt_ps[(dc, nq)] = ps_a.tile([128, 512], F32, tag="ps_a",
                                            name=f"qtps_{dc}_{nq}")
            ps = qt_ps[(dc, nq)]
            c = 2 * nq + h
            for j in range(J1):
                nc.tensor.matmul(
                    ps[:, h * 256:(h + 1) * 256],
                    lhsT=wqs[j][:, dc * 128:(dc + 1) * 128],
                    rhs=d1T[c][:, j, :],
                    start=(j == 0),
                    stop=(j == J1 - 1),
                )

        def qt_evict(dc, nq, sc_act, rs_act):
            ps = qt_ps[(dc, nq)]
            qt_bias_sc(ps, dc, nq, sc_act)
            qt_bias_rs(ps, dc, nq, rs_act)

        def qt_unit(dc, nq, sc_act, rs_act):
            qt_half(dc, nq, 0)
            qt_half(dc, nq, 1)
            qt_evict(dc, nq, sc_act, rs_act)

        # ---------------- residual Q via xbar DMA transpose ------------------
        # qres3[qg][q_low, j, dc, c] = Q[qg*512 + j*128 + q_low, dc*128 + c]
        qres3 = [big.tile([128, 4, 2, 128], BF16, tag=f"qres{qg}",
                          name=f"qres{qg}")
                 for qg in range(4)]

        def qres_xbar(qg, dc):
            nc.sync.dma_start_transpose(
                out=qres3[qg][:, :, dc, :],
                in_=qtbf[:, dc, qg * 512:(qg + 1) * 512],
            )

        # ---------------- scores + exp --------------------------------------
        expT = [
            [big.tile([128, 2, 1024], FP8, tag=f"expT{kp}_{nh}",
                      name=f"expT{kp}_{nh}")
             for nh in range(2)]
            for kp in range(KP)
        ]

        def scores_unit(km, nh):
            ps = ps_sc.tile([128, 1024], F32, tag="ps_sc")
            for half in range(2):
                nq = nh * 2 + half
                nc.tensor.matmul(
                    ps[:, half * 512:(half + 1) * 512],
                    lhsT=kt_sb[:, :, km * 128:(km + 1) * 128],
                    rhs=qt_sb[:, :, nq * 512:(nq + 1) * 512],
                    perf_mode=PM_DR,
                    start=True,
                    stop=True,
                )
            nc.scalar.activation(
                expT[km // 2][nh][:, km % 2, :], ps[:], AF.Exp, scale=SCALE
            )

        # ---------------- context + residual + out DMA ----------------------
        out_c = [stage.tile([128, 2 * D], F32, tag="outc", name=f"outc{c}")
                 for c in range(QB // 2)]
        ctxA = [big.tile([128, D + 1], BF16, tag=f"ctxA{i}", name=f"ctxA{i}")
                for i in range(8)]

        def ctx_mm(pc, qb, kp, start, stop):
            h, qq = qb // 8, qb % 8
            nc.tensor.matmul(
                pc,
                lhsT=expT[kp][h][:, :, qq * 128:(qq + 1) * 128],
                rhs=key2[kp][:],
                perf_mode=PM_DR,
                start=start,
                stop=stop,
            )

        def ctx_finish(pc, qb, div_act=False, add_dve=False):
            rc = small.tile([128, 1], F32, tag="recip")
            nc.vector.reciprocal(rc[:], pc[:, D:D + 1])
            c = qb // 2
            osl = out_c[c][:, (qb % 2) * D:(qb % 2 + 1) * D]
            if div_act:
                nc.scalar.activation(osl, pc[:, :D], AF.Identity, scale=rc[:])
            else:
                nc.vector.tensor_scalar(osl, pc[:, :D], rc[:], None, MULT)
            qg, j = qb // 4, qb % 4
            o2 = osl.rearrange("p (a b) -> p a b", a=2)
            if add_dve:
                nc.vector.tensor_tensor(o2, o2, qres3[qg][:, j, :, :], ADD)
            else:
                nc.gpsimd.tensor_add(o2, o2, qres3[qg][:, j, :, :])
            nc.sync.dma_start(
                out=out[qb * 128:(qb + 1) * 128, :],
                in_=osl,
            )

        ctx_pc = {}

        def ctx_part(qb, kp_lo, kp_hi, last):
            # partial kp-chain; PSUM accumulation persists across other work
            if kp_lo == 0:
                pc_full = ps_a.tile([128, 512], F32, tag="ps_a",
                                    name=f"ctxpc{qb}")
                ctx_pc[qb] = pc_full[:, :D + 1]
            pc = ctx_pc[qb]
            for kp in range(kp_lo, kp_hi):
                ctx_mm(pc, qb, kp, kp == 0, last and kp == kp_hi - 1)

        def ctx_h0_a(qb):
            ctx_part(qb, 0, 4, False)

        def ctx_h0_b(qb):
            ctx_part(qb, 4, KP, True)
            ctx_finish(ctx_pc[qb], qb)

        def ctx_a1(qb):
            ctx_part(qb, 0, 4, False)

        def ctx_a2(qb):
            ctx_part(qb, 4, KP_A, True)
            nc.vector.tensor_copy(ctxA[qb - 8][:], ctx_pc[qb])

        def ctx_h1_B(qb):
            pc_full = ps_a.tile([128, 512], F32, tag="ps_a")
            pc = pc_full[:, :D + 1]
            for kp in range(KP_A, KP):
                ctx_mm(pc, qb, kp, kp == KP_A, kp == KP - 1)
            nc.vector.tensor_tensor(pc, pc, ctxA[qb - 8][:], ADD)
            ctx_finish(pc, qb, div_act=True, add_dve=(qb % 4 >= 2))


        # ================= emission schedule ================================
        def units(fn, idxs):
            return [lambda i=i: fn(*i) if isinstance(i, tuple) else fn(i)
                    for i in idxs]

        def interleave(a, b, ratio):
            a = list(a)
            b = list(b)
            ia = ib = 0
            credit = 0.0
            while ia < len(a) or ib < len(b):
                if ia < len(a):
                    a[ia]()
                    ia += 1
                credit += ratio
                while credit >= 1.0 and ib < len(b):
                    b[ib]()
                    ib += 1
                    credit -= 1.0
            while ib < len(b):
                b[ib]()
                ib += 1

        # --- phase 1: QT nq0/nq1 chunk-paced as the first d1 chunks land
        #     (d1 loads first; no PE gap so the p-state stays hot), then
        #     KT nk0 the moment d2T lands.  Scores-path evicts split
        #     ACT (dc0) / DVE (dc1). ---
        for nq in range(2):
            for h in range(2):
                qt_half(0, nq, h)
                qt_half(1, nq, h)
            qt_evict(0, nq, sc_act=True, rs_act=False)
            qt_evict(1, nq, sc_act=False, rs_act=True)
        for dc in range(2):
            kt_unit(dc, 0, on_act=(dc == 0))
        for qg in range(2):
            for dc in range(2):
                qres_xbar(qg, dc)

        # --- phase 2: scores-h0 interleaved with QT nq2/3 + key transposes ---
        def qres_late(qg):
            for dc in range(2):
                qres_xbar(qg, dc)

        filler = (
            [lambda: kt_unit(0, 1, False), lambda: kt_unit(1, 1, False)]
            + units(key_tr, [0])
            + [lambda: kt_unit(0, 2, False), lambda: kt_unit(1, 2, False),
               lambda: kt_unit(0, 3, False), lambda: kt_unit(1, 3, False)]
            + units(key_tr, [1, 2, 3])
            + [lambda: qt_unit(0, 2, False, False),
               lambda: qt_unit(1, 2, False, False)]
            + units(key_tr, [4, 5])
            + [lambda: qt_unit(0, 3, False, False),
               lambda: qt_unit(1, 3, False, False)]
            + units(key_tr, [6, 7])
            + [lambda: qres_late(2), lambda: qres_late(3)]
        )
        scores_h0 = units(scores_unit, [(km, 0) for km in range(KB)])
        interleave(scores_h0, filler, len(filler) / len(scores_h0))

        # --- phase 3: scores-h1 with ctx-h0 and ctx-h1-A ---
        # ctx_h1_A reads exps km0-13 of h1, so A units may only be emitted
        # after scores-h1 km13 (Tile orders by emission-time dependencies)
        sc_h1_a = units(scores_unit, [(km, 1) for km in range(2 * KP_A)])
        sc_h1_b = units(scores_unit, [(km, 1) for km in range(2 * KP_A, KB)])
        ctx0 = [f for qb in range(0, 8)
                for f in (lambda q=qb: ctx_h0_a(q), lambda q=qb: ctx_h0_b(q))]
        ctxa = [f for qb in range(8, 16)
                for f in (lambda q=qb: ctx_a1(q), lambda q=qb: ctx_a2(q))]
        for u in sc_h1_a[:4]:
            u()
        interleave(sc_h1_a[4:], ctx0, len(ctx0) / len(sc_h1_a[4:]))
        for u in sc_h1_b:
            u()
        for u in ctxa:
            u()

        # --- phase 4: ctx-h1-B tail ---
        for qb in range(8, 16):
            ctx_h1_B(qb)

    nc.compile()
    return nc


_NC = None
_last_in_maps = None


def make_host_inputs(data1_b, data2_b, Wq, bq, Wk, bk):
    """Pack one batch element's inputs into the device layout (bf16 + f32).

    d1t[n, p, j, q'] = data1[n*512 + q', 8p + j]   (q-chunked, i p-major)
    d2t[p, j, k]     = data2[k, 2p + j]
    wq[p, j*256+d]   = Wq[8p + j, d]; wk[p, j*256+d] = Wk[2p + j, d]
    """
    bf = ml_dtypes.bfloat16
    a1 = np.asarray(data1_b, np.float32).astype(bf)      # [2048, 1024]
    d1t = np.ascontiguousarray(
        a1.reshape(8, 256, 128, J1).transpose(0, 2, 3, 1))
    f8 = ml_dtypes.float8_e4m3
    a2 = np.asarray(data2_b, np.float32).astype(f8)      # [2048, 256]
    d2t = np.ascontiguousarray(
        a2.reshape(LK, 128, J2).transpose(1, 2, 0))
    Wq = np.asarray(Wq, dtype=np.float32)
    Wk = np.asarray(Wk, dtype=np.float32)
    bq = np.asarray(bq, dtype=np.float32)
    bk = np.asarray(bk, dtype=np.float32)
    wq = np.ascontiguousarray(
        Wq.astype(bf).reshape(128, J1, D).reshape(128, J1 * D))
    wk = np.ascontiguousarray(
        Wk.astype(f8).reshape(128, J2, D).reshape(128, J2 * D))
    bias = np.empty((128, 4), np.float32)
    bqk = bq + bk
    for c in range(2):
        bias[:, c] = bq[c * 128:(c + 1) * 128]
        bias[:, 2 + c] = bqk[c * 128:(c + 1) * 128]
    return {"d1t": d1t, "d2t": d2t, "wq": wq, "wk": wk, "bias": bias}


def _get_nc():
    global _NC
    if _NC is None:
        _NC = _build()
    return _NC


def kernel(data1, data2, Wq, bq, Wk, bk):
    global _last_in_maps
    data1 = np.asarray(data1, dtype=np.float32)
    data2 = np.asarray(data2, dtype=np.float32)

    nc = _get_nc()
    shared = None
    in_maps = []
    for b in range(B):
        m = make_host_inputs(data1[b], data2[b], Wq, bq, Wk, bk)
        if shared is None:
            shared = {k: m[k] for k in ("wq", "wk", "bias")}
        m.update(shared)
        in_maps.append(m)
    _last_in_maps = in_maps
    res = run_bass_kernel_spmd(nc, in_maps, core_ids=list(range(N_CORES)))
    return np.stack([res.results[i]["out"] for i in range(B)], axis=0)


# revision 19
# speedup vs baseline: 1.1823x; 1.1823x over previous
"""CoAttention kernel for Trainium2, data-parallel over batch across 8 NeuronCores.

Per core (one batch element b):
    query = data1[b] @ Wq + bq                      # [2048, 256]
    key   = data2[b] @ Wk + bk                      # [2048, 256]
    attn  = softmax(SCALE * query @ key^T)          # row-constant terms cancel
    out   = attn @ key + query

Device-side strategy (v7):
  - The host uploads d1^T and d2^T in bf16, i-interleaved p-major so
    every DMA descriptor is one 8KB contiguous run per partition, and
    d1^T is split into four q-chunks so each QT projection unit
    completes as its chunk lands (the weight rows are packed in the
    matching i-permutation; contraction order is free).  No device
    casts, no input transposes, ~5.6 MiB/core input traffic.
  - softmax(q@(k+bk)^T) drops bias terms constant along k, and
    sum(attn)==1 makes attn@(key+bk) == attn@key + bk, so the key value
    matrix carries NO bias; bq biases the scores path and (bq+bk) the
    residual path from the same QT PSUM.  The softmax denominator is a
    memset 1.0 column appended to the fp8 key values.
  - Scores-path QT evicts split ACT/DVE so exp #0 fires ~1us after the
    second d1 chunk lands.  key values come from fp8 PE transposes of
    kt (stride-2 PSUM), not a second projection matmul.
  - Residual Q reaches [q, d] layout via xbar DMA transposes on the
    idle mid-kernel DMA engines: no PE, no PSUM, no DVE eviction.
  - scoresT [k, q] orientation lets exp(scoresT) feed the context
    matmul as the stationary operand; scores and context run in fp8e4m3
    DoubleRow.  ctx for the second q-half is split kp0-6 (inside the
    exp stream, evicted to bf16) + kp7 (after the last exp); post-exp
    divides run on the then-idle ACT engine (activation scale=recip AP).
  - Output is written in 8 chunks of 256 rows as each completes.
"""

import sys

if "/opt/trn_rl_repo" not in sys.path:
    sys.path.insert(0, "/opt/trn_rl_repo")

from contextlib import ExitStack

import ml_dtypes
import numpy as np

import concourse.bass as bass  # noqa: F401
import concourse.mybir as mybir
import concourse.tile as tile
from concourse import bacc
from concourse.bass_utils import run_bass_kernel_spmd

B, LQ, LK, DIN, D = 8, 2048, 2048, 1024, 256
N_CORES = 8
SCALE = float(1.0 / np.sqrt(1024.0).astype(np.float32))

BF16 = mybir.dt.bfloat16
FP8 = mybir.dt.float8e4
F32 = mybir.dt.float32
AF = mybir.ActivationFunctionType
PM_DR = mybir.MatmulPerfMode.DoubleRow
ADD = mybir.AluOpType.add
MULT = mybir.AluOpType.mult

QB = 16           # q blocks of 128
KB = 16           # k blocks of 128
J1 = 8            # d1 i-interleave factor (1024 = 128 * 8)
J2 = 2            # d2 i-interleave factor (256 = 128 * 2)
KP = KB // 2      # 8 fp8 DoubleRow k-pairs
KP_A = 7          # h1 ctx kp-split: A = kp0-6 inside exp stream, B = kp7 after


def _build():
    nc = bacc.Bacc("TRN2", target_bir_lowering=False, debug=False)
    d1t = nc.dram_tensor("d1t", [8, 128, J1, 256], BF16, kind="ExternalInput").ap()
    d2t = nc.dram_tensor("d2t", [128, J2, LK], FP8, kind="ExternalInput").ap()
    wq_d = nc.dram_tensor("wq", [128, 2048], BF16, kind="ExternalInput").ap()
    wk_d = nc.dram_tensor("wk", [128, 512], FP8, kind="ExternalInput").ap()
    bias = nc.dram_tensor("bias", [128, 4], F32, kind="ExternalInput").ap()
    out = nc.dram_tensor("out", [LQ, D], F32, kind="ExternalOutput").ap()

    with tile.TileContext(nc) as tc, ExitStack() as ctx:
        const = ctx.enter_context(tc.tile_pool(name="const", bufs=1))
        big = ctx.enter_context(tc.tile_pool(name="big", bufs=1))
        stage = ctx.enter_context(tc.tile_pool(name="stage", bufs=3))
        small = ctx.enter_context(tc.tile_pool(name="small", bufs=4))
        ps_a = ctx.enter_context(tc.tile_pool(name="ps_a", bufs=4, space="PSUM"))
        ps_sc = ctx.enter_context(tc.tile_pool(name="ps_sc", bufs=2, space="PSUM"))

        # ---------------- constants / small state ---------------------------
        warm_src = const.tile([128, 512], BF16, tag="warm_src")
        nc.gpsimd.memset(warm_src[:], 0.0)
        dummy = const.tile([128, 1], F32, tag="dummy")
        # force the exp ACT table load at kernel start (otherwise it stalls
        # the first real exp by ~1.3us mid-stream)
        nc.scalar.activation(dummy[:], warm_src[:, 0:1], AF.Exp)

        key2 = [
            big.tile([128, 2, D + 1], FP8, tag=f"key2_{kp}", name=f"key2_{kp}")
            for kp in range(KP)
        ]
        for kp in range(KP):
            nc.gpsimd.memset(key2[kp][:, :, D:D + 1], 1.0)

        # ---------------- loads ---------------------------------------------
        wq_sb = const.tile([128, 2048], BF16, tag="wq_sb")
        wk_sb = const.tile([128, 512], FP8, tag="wk_sb")
        bias_sb = const.tile([128, 4], F32, tag="bias_sb")
        d2T = big.tile([128, J2, LK], FP8, tag="d2T")
        d1T = [big.tile([128, J1, 256], BF16, tag=f"d1T{n}", name=f"d1T{n}")
               for n in range(8)]

        nc.sync.dma_start(out=wq_sb[:], in_=wq_d)
        for n in range(2):
            nc.sync.dma_start(out=d1T[n][:], in_=d1t[n])
        nc.sync.dma_start(out=bias_sb[:], in_=bias)
        nc.sync.dma_start(out=wk_sb[:], in_=wk_d)
        for n in range(2, 4):
            nc.sync.dma_start(out=d1T[n][:], in_=d1t[n])
        nc.sync.dma_start(out=d2T[:], in_=d2t)
        for n in range(4, 8):
            nc.sync.dma_start(out=d1T[n][:], in_=d1t[n])

        # weight slices in the same i-permutation as the activations
        wqs = [wq_sb[:, j * D:(j + 1) * D] for j in range(J1)]
        wks = [wk_sb[:, j * D:(j + 1) * D] for j in range(J2)]
        bq_col = bias_sb[:, 0:2]
        bqk_col = bias_sb[:, 2:4]

        # ---------------- PE p-state warmup (also bridges the d2T wait) -----
        for w in range(7):
            pw = ps_a.tile([128, 512], F32, tag="ps_a", name=f"warm{w}")
            nc.tensor.matmul(pw[:], lhsT=warm_src[:, :128], rhs=warm_src[:],
                             start=True, stop=True)

        # ---------------- K^T fp8 DoubleRow layout [128, 2, k] --------------
        kt_sb = big.tile([128, 2, LK], FP8, tag="kt_sb")

        wk2 = wk_sb[:].rearrange("p (j d) -> p j d", j=J2)

        def kt_unit(dc, nk, on_act):
            ps = ps_a.tile([128, 512], F32, tag="ps_a")
            nc.tensor.matmul(
                ps[:],
                lhsT=wk2[:, :, dc * 128:(dc + 1) * 128],
                rhs=d2T[:, :, nk * 512:(nk + 1) * 512],
                perf_mode=PM_DR,
                start=True,
                stop=True,
            )
            o = kt_sb[:, dc, nk * 512:(nk + 1) * 512]
            if on_act:
                nc.scalar.copy(o, ps[:])
            else:
                nc.vector.tensor_copy(o, ps[:])

        # ---------------- key values via fp8 DoubleRow matmuls ---------------
        def key_tr(kp):
            ps = ps_a.tile([128, 512], F32, tag="ps_a")
            for s in range(2):
                kb = 2 * kp + s
                nc.tensor.matmul(
                    ps[:, s * D:(s + 1) * D],
                    lhsT=d2T[:, :, kb * 128:(kb + 1) * 128],
                    rhs=wk2,
                    perf_mode=PM_DR,
                    start=True,
                    stop=True,
                )
            nc.vector.tensor_copy(
                key2[kp][:, :, :D],
                ps[:].rearrange("p (s d) -> p s d", s=2),
            )

        # ---------------- QT projection ------------------------------------
        qt_sb = big.tile([128, 2, LQ], FP8, tag="qt_sb")
        qtbf = big.tile([128, 2, LQ], BF16, tag="qtbf")

        def qt_bias_sc(ps, dc, nq, on_act):
            o = qt_sb[:, dc, nq * 512:(nq + 1) * 512]
            if on_act:
                nc.scalar.activation(o, ps[:], AF.Identity,
                                     bias=bq_col[:, dc:dc + 1])
            else:
                nc.vector.tensor_scalar(o, ps[:], bq_col[:, dc:dc + 1], None, ADD)

        def qt_bias_rs(ps, dc, nq, on_act):
            o = qtbf[:, dc, nq * 512:(nq + 1) * 512]
            if on_act:
                nc.scalar.activation(o, ps[:], AF.Identity,
                                     bias=bqk_col[:, dc:dc + 1])
            else:
                nc.vector.tensor_scalar(o, ps[:], bqk_col[:, dc:dc + 1], None, ADD)

        qt_ps = {}

        def qt_half(dc, nq, h):
            # one 8-chain over q-chunk c = 2*nq + h into half of the psum tile
            if h == 0:
                qt_ps[(dc, nq)] = ps_a.tile([128, 512], F32, tag="ps_a",
                                            name=f"qtps_{dc}_{nq}")
            ps = qt_ps[(dc, nq)]
            c = 2 * nq + h
            for j in range(J1):
                nc.tensor.matmul(
                    ps[:, h * 256:(h + 1) * 256],
                    lhsT=wqs[j][:, dc * 128:(dc + 1) * 128],
                    rhs=d1T[c][:, j, :],
                    start=(j == 0),
                    stop=(j == J1 - 1),
                )

        def qt_evict(dc, nq, sc_act, rs_act):
            ps = qt_ps[(dc, nq)]
            qt_bias_sc(ps, dc, nq, sc_act)
            qt_bias_rs(ps, dc, nq, rs_act)

        def qt_unit(dc, nq, sc_act, rs_act):
            qt_half(dc, nq, 0)
            qt_half(dc, nq, 1)
            qt_evict(dc, nq, sc_act, rs_act)

        # ---------------- residual Q via xbar DMA transpose ------------------
        # qres3[qg][q_low, j, dc, c] = Q[qg*512 + j*128 + q_low, dc*128 + c]
        qres3 = [big.tile([128, 4, 2, 128], BF16, tag=f"qres{qg}",
                          name=f"qres{qg}")
                 for qg in range(4)]

        def qres_xbar(qg, dc):
            nc.sync.dma_start_transpose(
                out=qres3[qg][:, :, dc, :],
                in_=qtbf[:, dc, qg * 512:(qg + 1) * 512],
            )

        # ---------------- scores + exp --------------------------------------
        expT = [
            [big.tile([128, 2, 1024], FP8, tag=f"expT{kp}_{nh}",
                      name=f"expT{kp}_{nh}")
             for nh in range(2)]
            for kp in range(KP)
        ]

        def scores_unit(km, nh):
            ps = ps_sc.tile([128, 1024], F32, tag="ps_sc")
            for half in range(2):
                nq = nh * 2 + half
                nc.tensor.matmul(
                    ps[:, half * 512:(half + 1) * 512],
                    lhsT=kt_sb[:, :, km * 128:(km + 1) * 128],
                    rhs=qt_sb[:, :, nq * 512:(nq + 1) * 512],
                    perf_mode=PM_DR,
                    start=True,
                    stop=True,
                )
            nc.scalar.activation(
                expT[km // 2][nh][:, km % 2, :], ps[:], AF.Exp, scale=SCALE
            )

        # ---------------- context + residual + out DMA ----------------------
        out_c = [stage.tile([128, 2 * D], F32, tag="outc", name=f"outc{c}")
                 for c in range(QB // 2)]
        ctxA = [big.tile([128, D + 1], BF16, tag=f"ctxA{i}", name=f"ctxA{i}")
                for i in range(8)]

        def ctx_mm(pc, qb, kp, start, stop):
            h, qq = qb // 8, qb % 8
            nc.tensor.matmul(
                pc,
                lhsT=expT[kp][h][:, :, qq * 128:(qq + 1) * 128],
                rhs=key2[kp][:],
                perf_mode=PM_DR,
                start=start,
                stop=stop,
            )

        def ctx_finish(pc, qb, div_act=False, add_dve=False):
            rc = small.tile([128, 1], F32, tag="recip")
            nc.vector.reciprocal(rc[:], pc[:, D:D + 1])
            c = qb // 2
            osl = out_c[c][:, (qb % 2) * D:(qb % 2 + 1) * D]
            if div_act:
                nc.scalar.activation(osl, pc[:, :D], AF.Identity, scale=rc[:])
            else:
                nc.vector.tensor_scalar(osl, pc[:, :D], rc[:], None, MULT)
            qg, j = qb // 4, qb % 4
            o2 = osl.rearrange("p (a b) -> p a b", a=2)
            if add_dve:
                nc.vector.tensor_tensor(o2, o2, qres3[qg][:, j, :, :], ADD)
            else:
                nc.gpsimd.tensor_add(o2, o2, qres3[qg][:, j, :, :])
            if qb % 2 == 1:
                nc.sync.dma_start(
                    out=out[c * 256:(c + 1) * 256, :].rearrange(
                        "(t p) d -> p t d", p=128),
                    in_=out_c[c][:].rearrange("p (t d) -> p t d", d=D),
                )

        def ctx_unit_h0(qb):
            pc_full = ps_a.tile([128, 512], F32, tag="ps_a")
            pc = pc_full[:, :D + 1]
            for kp in range(KP):
                ctx_mm(pc, qb, kp, kp == 0, kp == KP - 1)
            ctx_finish(pc, qb)

        def ctx_h1_A(qb):
            pc_full = ps_a.tile([128, 512], F32, tag="ps_a")
            pc = pc_full[:, :D + 1]
            for kp in range(KP_A):
                ctx_mm(pc, qb, kp, kp == 0, kp == KP_A - 1)
            nc.vector.tensor_copy(ctxA[qb - 8][:], pc)

        def ctx_h1_B(qb):
            pc_full = ps_a.tile([128, 512], F32, tag="ps_a")
            pc = pc_full[:, :D + 1]
            for kp in range(KP_A, KP):
                ctx_mm(pc, qb, kp, kp == KP_A, kp == KP - 1)
            nc.vector.tensor_tensor(pc, pc, ctxA[qb - 8][:], ADD)
            ctx_finish(pc, qb, div_act=True, add_dve=(qb % 4 >= 2))


        # ================= emission schedule ================================
        def units(fn, idxs):
            return [lambda i=i: fn(*i) if isinstance(i, tuple) else fn(i)
                    for i in idxs]

        def interleave(a, b, ratio):
            a = list(a)
            b = list(b)
            ia = ib = 0
            credit = 0.0
            while ia < len(a) or ib < len(b):
                if ia < len(a):
                    a[ia]()
                    ia += 1
                credit += ratio
                while credit >= 1.0 and ib < len(b):
                    b[ib]()
                    ib += 1
                    credit -= 1.0
            while ib < len(b):
                b[ib]()
                ib += 1

        # --- phase 1: QT nq0/nq1 chunk-paced as the first d1 chunks land
        #     (d1 loads first; no PE gap so the p-state stays hot), then
        #     KT nk0 the moment d2T lands.  Scores-path evicts split
        #     ACT (dc0) / DVE (dc1). ---
        for nq in range(2):
            for h in range(2):
                qt_half(0, nq, h)
                qt_half(1, nq, h)
            qt_evict(0, nq, sc_act=True, rs_act=False)
            qt_evict(1, nq, sc_act=False, rs_act=True)
        for dc in range(2):
            kt_unit(dc, 0, on_act=(dc == 0))
        for qg in range(2):
            for dc in range(2):
                qres_xbar(qg, dc)

        # --- phase 2: scores-h0 interleaved with QT nq2/3 + key transposes ---
        def qres_late(qg):
            for dc in range(2):
                qres_xbar(qg, dc)

        filler = (
            [lambda: kt_unit(0, 1, False), lambda: kt_unit(1, 1, False)]
            + units(key_tr, [0])
            + [lambda: kt_unit(0, 2, False), lambda: kt_unit(1, 2, False),
               lambda: kt_unit(0, 3, False), lambda: kt_unit(1, 3, False)]
            + units(key_tr, [1, 2, 3])
            + [lambda: qt_unit(0, 2, False, False),
               lambda: qt_unit(1, 2, False, False)]
            + units(key_tr, [4, 5])
            + [lambda: qt_unit(0, 3, False, False),
               lambda: qt_unit(1, 3, False, False)]
            + units(key_tr, [6, 7])
            + [lambda: qres_late(2), lambda: qres_late(3)]
        )
        scores_h0 = units(scores_unit, [(km, 0) for km in range(KB)])
        interleave(scores_h0, filler, len(filler) / len(scores_h0))

        # --- phase 3: scores-h1 with ctx-h0 and ctx-h1-A ---
        # ctx_h1_A reads exps km0-13 of h1, so A units may only be emitted
        # after scores-h1 km13 (Tile orders by emission-time dependencies)
        sc_h1_a = units(scores_unit, [(km, 1) for km in range(2 * KP_A)])
        sc_h1_b = units(scores_unit, [(km, 1) for km in range(2 * KP_A, KB)])
        ctx0 = units(ctx_unit_h0, list(range(0, 8)))
        ctxa = units(ctx_h1_A, list(range(8, 16)))
        for u in sc_h1_a[:5]:
            u()
        interleave(sc_h1_a[5:], ctx0, len(ctx0) / len(sc_h1_a[5:]))
        for u in sc_h1_b:
            u()
        for u in ctxa:
            u()

        # --- phase 4: ctx-h1-B tail ---
        for qb in range(8, 16):
            ctx_h1_B(qb)

    nc.compile()
    return nc


_NC = None
_last_in_maps = None


def make_host_inputs(data1_b, data2_b, Wq, bq, Wk, bk):
    """Pack one batch element's inputs into the device layout (bf16 + f32).

    d1t[n, p, j, q'] = data1[n*512 + q', 8p + j]   (q-chunked, i p-major)
    d2t[p, j, k]     = data2[k, 2p + j]
    wq[p, j*256+d]   = Wq[8p + j, d]; wk[p, j*256+d] = Wk[2p + j, d]
    """
    bf = ml_dtypes.bfloat16
    a1 = np.asarray(data1_b, np.float32).astype(bf)      # [2048, 1024]
    d1t = np.ascontiguousarray(
        a1.reshape(8, 256, 128, J1).transpose(0, 2, 3, 1))
    f8 = ml_dtypes.float8_e4m3
    a2 = np.asarray(data2_b, np.float32).astype(f8)      # [2048, 256]
    d2t = np.ascontiguousarray(
        a2.reshape(LK, 128, J2).transpose(1, 2, 0))
    Wq = np.asarray(Wq, dtype=np.float32)
    Wk = np.asarray(Wk, dtype=np.float32)
    bq = np.asarray(bq, dtype=np.float32)
    bk = np.asarray(bk, dtype=np.float32)
    wq = np.ascontiguousarray(
        Wq.astype(bf).reshape(128, J1, D).reshape(128, J1 * D))
    wk = np.ascontiguousarray(
        Wk.astype(f8).reshape(128, J2, D).reshape(128, J2 * D))
    bias = np.empty((128, 4), np.float32)
    bqk = bq + bk
    for c in range(2):
        bias[:, c] = bq[c * 128:(c + 1) * 128]
        bias[:, 2 + c] = bqk[c * 128:(c + 1) * 128]
    return {"d1t": d1t, "d2t": d2t, "wq": wq, "wk": wk, "bias": bias}


def _get_nc():
    global _NC
    if _NC is None:
        _NC = _build()
    return _NC


def kernel(data1, data2, Wq, bq, Wk, bk):
    global _last_in_maps
    data1 = np.asarray(data1, dtype=np.float32)
    data2 = np.asarray(data2, dtype=np.float32)

    nc = _get_nc()
    shared = None
    in_maps = []
    for b in range(B):
        m = make_host_inputs(data1[b], data2[b], Wq, bq, Wk, bk)
        if shared is None:
            shared = {k: m[k] for k in ("wq", "wk", "bias")}
        m.update(shared)
        in_maps.append(m)
    _last_in_maps = in_maps
    res = run_bass_kernel_spmd(nc, in_maps, core_ids=list(range(N_CORES)))
    return np.stack([res.results[i]["out"] for i in range(B)], axis=0)


# revision 20
# speedup vs baseline: 1.2026x; 1.0171x over previous
"""CoAttention kernel for Trainium2, data-parallel over batch across 8 NeuronCores.

Per core (one batch element b):
    query = data1[b] @ Wq + bq                      # [2048, 256]
    key   = data2[b] @ Wk + bk                      # [2048, 256]
    attn  = softmax(SCALE * query @ key^T)          # row-constant terms cancel
    out   = attn @ key + query

Device-side strategy (v7):
  - The host uploads d1^T and d2^T in bf16, i-interleaved p-major so
    every DMA descriptor is one 8KB contiguous run per partition, and
    d1^T is split into four q-chunks so each QT projection unit
    completes as its chunk lands (the weight rows are packed in the
    matching i-permutation; contraction order is free).  No device
    casts, no input transposes, ~5.6 MiB/core input traffic.
  - softmax(q@(k+bk)^T) drops bias terms constant along k, and
    sum(attn)==1 makes attn@(key+bk) == attn@key + bk, so the key value
    matrix carries NO bias; bq biases the scores path and (bq+bk) the
    residual path from the same QT PSUM.  The softmax denominator is a
    memset 1.0 column appended to the fp8 key values.
  - Scores-path QT evicts split ACT/DVE so exp #0 fires ~1us after the
    second d1 chunk lands.  key values come from fp8 PE transposes of
    kt (stride-2 PSUM), not a second projection matmul.
  - Residual Q reaches [q, d] layout via xbar DMA transposes on the
    idle mid-kernel DMA engines: no PE, no PSUM, no DVE eviction.
  - scoresT [k, q] orientation lets exp(scoresT) feed the context
    matmul as the stationary operand; scores and context run in fp8e4m3
    DoubleRow.  ctx for the second q-half is split kp0-6 (inside the
    exp stream, evicted to bf16) + kp7 (after the last exp); post-exp
    divides run on the then-idle ACT engine (activation scale=recip AP).
  - Output is written in 8 chunks of 256 rows as each completes.
"""

import sys

if "/opt/trn_rl_repo" not in sys.path:
    sys.path.insert(0, "/opt/trn_rl_repo")

from contextlib import ExitStack

import ml_dtypes
import numpy as np

import concourse.bass as bass  # noqa: F401
import concourse.mybir as mybir
import concourse.tile as tile
from concourse import bacc
from concourse.bass_utils import run_bass_kernel_spmd

B, LQ, LK, DIN, D = 8, 2048, 2048, 1024, 256
N_CORES = 8
SCALE = float(1.0 / np.sqrt(1024.0).astype(np.float32))

BF16 = mybir.dt.bfloat16
FP8 = mybir.dt.float8e4
F32 = mybir.dt.float32
AF = mybir.ActivationFunctionType
PM_DR = mybir.MatmulPerfMode.DoubleRow
ADD = mybir.AluOpType.add
MULT = mybir.AluOpType.mult

QB = 16           # q blocks of 128
KB = 16           # k blocks of 128
J1 = 8            # d1 i-interleave factor (1024 = 128 * 8)
J2 = 2            # d2 i-interleave factor (256 = 128 * 2)
KP = KB // 2      # 8 fp8 DoubleRow k-pairs
KP_A = 7          # h1 ctx kp-split: A = kp0-6 inside exp stream, B = kp7 after


def _build():
    nc = bacc.Bacc("TRN2", target_bir_lowering=False, debug=False)
    d1t = nc.dram_tensor("d1t", [8, 128, J1, 256], BF16, kind="ExternalInput").ap()
    d2t = nc.dram_tensor("d2t", [128, J2, LK], FP8, kind="ExternalInput").ap()
    wq_d = nc.dram_tensor("wq", [128, 2048], BF16, kind="ExternalInput").ap()
    wk_d = nc.dram_tensor("wk", [128, 512], FP8, kind="ExternalInput").ap()
    bias = nc.dram_tensor("bias", [128, 4], F32, kind="ExternalInput").ap()
    out = nc.dram_tensor("out", [LQ, D], F32, kind="ExternalOutput").ap()

    with tile.TileContext(nc) as tc, ExitStack() as ctx:
        const = ctx.enter_context(tc.tile_pool(name="const", bufs=1))
        big = ctx.enter_context(tc.tile_pool(name="big", bufs=1))
        stage = ctx.enter_context(tc.tile_pool(name="stage", bufs=3))
        small = ctx.enter_context(tc.tile_pool(name="small", bufs=4))
        ps_a = ctx.enter_context(tc.tile_pool(name="ps_a", bufs=4, space="PSUM"))
        ps_sc = ctx.enter_context(tc.tile_pool(name="ps_sc", bufs=2, space="PSUM"))

        # ---------------- constants / small state ---------------------------
        warm_src = const.tile([128, 512], BF16, tag="warm_src")
        nc.gpsimd.memset(warm_src[:], 0.0)
        dummy = const.tile([128, 1], F32, tag="dummy")
        # force the exp ACT table load at kernel start (otherwise it stalls
        # the first real exp by ~1.3us mid-stream)
        nc.scalar.activation(dummy[:], warm_src[:, 0:1], AF.Exp)

        key2 = [
            big.tile([128, 2, D + 1], FP8, tag=f"key2_{kp}", name=f"key2_{kp}")
            for kp in range(KP)
        ]
        for kp in range(KP):
            nc.gpsimd.memset(key2[kp][:, :, D:D + 1], 1.0)

        # ---------------- loads ---------------------------------------------
        wq_sb = const.tile([128, 2048], BF16, tag="wq_sb")
        wk_sb = const.tile([128, 512], FP8, tag="wk_sb")
        bias_sb = const.tile([128, 4], F32, tag="bias_sb")
        d2T = big.tile([128, J2, LK], FP8, tag="d2T")
        d1T = [big.tile([128, J1, 256], BF16, tag=f"d1T{n}", name=f"d1T{n}")
               for n in range(8)]

        nc.sync.dma_start(out=wq_sb[:], in_=wq_d)
        for n in range(2):
            nc.sync.dma_start(out=d1T[n][:], in_=d1t[n])
        nc.sync.dma_start(out=bias_sb[:], in_=bias)
        nc.sync.dma_start(out=wk_sb[:], in_=wk_d)
        for n in range(2, 4):
            nc.sync.dma_start(out=d1T[n][:], in_=d1t[n])
        nc.sync.dma_start(out=d2T[:], in_=d2t)
        for n in range(4, 8):
            nc.sync.dma_start(out=d1T[n][:], in_=d1t[n])

        # weight slices in the same i-permutation as the activations
        wqs = [wq_sb[:, j * D:(j + 1) * D] for j in range(J1)]
        wks = [wk_sb[:, j * D:(j + 1) * D] for j in range(J2)]
        bq_col = bias_sb[:, 0:2]
        bqk_col = bias_sb[:, 2:4]

        # ---------------- PE p-state warmup (also bridges the d2T wait) -----
        for w in range(7):
            pw = ps_a.tile([128, 512], F32, tag="ps_a", name=f"warm{w}")
            nc.tensor.matmul(pw[:], lhsT=warm_src[:, :128], rhs=warm_src[:],
                             start=True, stop=True)

        # ---------------- K^T fp8 DoubleRow layout [128, 2, k] --------------
        kt_sb = big.tile([128, 2, LK], FP8, tag="kt_sb")

        wk2 = wk_sb[:].rearrange("p (j d) -> p j d", j=J2)

        def kt_unit(dc, nk, on_act):
            ps = ps_a.tile([128, 512], F32, tag="ps_a")
            nc.tensor.matmul(
                ps[:],
                lhsT=wk2[:, :, dc * 128:(dc + 1) * 128],
                rhs=d2T[:, :, nk * 512:(nk + 1) * 512],
                perf_mode=PM_DR,
                start=True,
                stop=True,
            )
            o = kt_sb[:, dc, nk * 512:(nk + 1) * 512]
            if on_act:
                nc.scalar.copy(o, ps[:])
            else:
                nc.vector.tensor_copy(o, ps[:])

        # ---------------- key values via fp8 DoubleRow matmuls ---------------
        def key_tr(kp):
            ps = ps_a.tile([128, 512], F32, tag="ps_a")
            for s in range(2):
                kb = 2 * kp + s
                nc.tensor.matmul(
                    ps[:, s * D:(s + 1) * D],
                    lhsT=d2T[:, :, kb * 128:(kb + 1) * 128],
                    rhs=wk2,
                    perf_mode=PM_DR,
                    start=True,
                    stop=True,
                )
            nc.vector.tensor_copy(
                key2[kp][:, :, :D],
                ps[:].rearrange("p (s d) -> p s d", s=2),
            )

        # ---------------- QT projection ------------------------------------
        qt_sb = big.tile([128, 2, LQ], FP8, tag="qt_sb")
        qtbf = big.tile([128, 2, LQ], BF16, tag="qtbf")

        def qt_bias_sc(ps, dc, nq, on_act):
            o = qt_sb[:, dc, nq * 512:(nq + 1) * 512]
            if on_act:
                nc.scalar.activation(o, ps[:], AF.Identity,
                                     bias=bq_col[:, dc:dc + 1])
            else:
                nc.vector.tensor_scalar(o, ps[:], bq_col[:, dc:dc + 1], None, ADD)

        def qt_bias_rs(ps, dc, nq, on_act):
            o = qtbf[:, dc, nq * 512:(nq + 1) * 512]
            if on_act:
                nc.scalar.activation(o, ps[:], AF.Identity,
                                     bias=bqk_col[:, dc:dc + 1])
            else:
                nc.vector.tensor_scalar(o, ps[:], bqk_col[:, dc:dc + 1], None, ADD)

        qt_ps = {}

        def qt_half(dc, nq, h):
            # one 8-chain over q-chunk c = 2*nq + h into half of the psum tile
            if h == 0:
                qt_ps[(dc, nq)] = ps_a.tile([128, 512], F32, tag="ps_a",
                                            name=f"qtps_{dc}_{nq}")
            ps = qt_ps[(dc, nq)]
            c = 2 * nq + h
            for j in range(J1):
                nc.tensor.matmul(
                    ps[:, h * 256:(h + 1) * 256],
                    lhsT=wqs[j][:, dc * 128:(dc + 1) * 128],
                    rhs=d1T[c][:, j, :],
                    start=(j == 0),
                    stop=(j == J1 - 1),
                )

        def qt_evict(dc, nq, sc_act, rs_act):
            ps = qt_ps[(dc, nq)]
            qt_bias_sc(ps, dc, nq, sc_act)
            qt_bias_rs(ps, dc, nq, rs_act)

        def qt_unit(dc, nq, sc_act, rs_act):
            qt_half(dc, nq, 0)
            qt_half(dc, nq, 1)
            qt_evict(dc, nq, sc_act, rs_act)

        # ---------------- residual Q via xbar DMA transpose ------------------
        # qres3[qg][q_low, j, dc, c] = Q[qg*512 + j*128 + q_low, dc*128 + c]
        qres3 = [big.tile([128, 4, 2, 128], BF16, tag=f"qres{qg}",
                          name=f"qres{qg}")
                 for qg in range(4)]

        def qres_xbar(qg, dc):
            nc.sync.dma_start_transpose(
                out=qres3[qg][:, :, dc, :],
                in_=qtbf[:, dc, qg * 512:(qg + 1) * 512],
            )

        # ---------------- scores + exp --------------------------------------
        expT = [
            [big.tile([128, 2, 1024], FP8, tag=f"expT{kp}_{nh}",
                      name=f"expT{kp}_{nh}")
             for nh in range(2)]
            for kp in range(KP)
        ]

        def scores_unit(km, nh):
            ps = ps_sc.tile([128, 1024], F32, tag="ps_sc")
            for half in range(2):
                nq = nh * 2 + half
                nc.tensor.matmul(
                    ps[:, half * 512:(half + 1) * 512],
                    lhsT=kt_sb[:, :, km * 128:(km + 1) * 128],
                    rhs=qt_sb[:, :, nq * 512:(nq + 1) * 512],
                    perf_mode=PM_DR,
                    start=True,
                    stop=True,
                )
            nc.scalar.activation(
                expT[km // 2][nh][:, km % 2, :], ps[:], AF.Exp, scale=SCALE
            )

        # ---------------- context + residual + out DMA ----------------------
        out_c = [stage.tile([128, 2 * D], F32, tag="outc", name=f"outc{c}")
                 for c in range(QB // 2)]
        ctxA = [big.tile([128, D + 1], BF16, tag=f"ctxA{i}", name=f"ctxA{i}")
                for i in range(8)]

        def ctx_mm(pc, qb, kp, start, stop):
            h, qq = qb // 8, qb % 8
            nc.tensor.matmul(
                pc,
                lhsT=expT[kp][h][:, :, qq * 128:(qq + 1) * 128],
                rhs=key2[kp][:],
                perf_mode=PM_DR,
                start=start,
                stop=stop,
            )

        def ctx_finish(pc, qb, div_act=False, add_dve=False):
            rc = small.tile([128, 1], F32, tag="recip")
            nc.vector.reciprocal(rc[:], pc[:, D:D + 1])
            c = qb // 2
            osl = out_c[c][:, (qb % 2) * D:(qb % 2 + 1) * D]
            if div_act:
                nc.scalar.activation(osl, pc[:, :D], AF.Identity, scale=rc[:])
            else:
                nc.vector.tensor_scalar(osl, pc[:, :D], rc[:], None, MULT)
            qg, j = qb // 4, qb % 4
            o2 = osl.rearrange("p (a b) -> p a b", a=2)
            if add_dve:
                nc.vector.tensor_tensor(o2, o2, qres3[qg][:, j, :, :], ADD)
            else:
                nc.gpsimd.tensor_add(o2, o2, qres3[qg][:, j, :, :])
            if qb % 2 == 1:
                nc.sync.dma_start(
                    out=out[c * 256:(c + 1) * 256, :].rearrange(
                        "(t p) d -> p t d", p=128),
                    in_=out_c[c][:].rearrange("p (t d) -> p t d", d=D),
                )

        def ctx_unit_h0(qb):
            pc_full = ps_a.tile([128, 512], F32, tag="ps_a")
            pc = pc_full[:, :D + 1]
            for kp in range(KP):
                ctx_mm(pc, qb, kp, kp == 0, kp == KP - 1)
            ctx_finish(pc, qb)

        def ctx_h1_A(qb):
            pc_full = ps_a.tile([128, 512], F32, tag="ps_a")
            pc = pc_full[:, :D + 1]
            for kp in range(KP_A):
                ctx_mm(pc, qb, kp, kp == 0, kp == KP_A - 1)
            nc.vector.tensor_copy(ctxA[qb - 8][:], pc)

        def ctx_h1_B(qb):
            pc_full = ps_a.tile([128, 512], F32, tag="ps_a")
            pc = pc_full[:, :D + 1]
            for kp in range(KP_A, KP):
                ctx_mm(pc, qb, kp, kp == KP_A, kp == KP - 1)
            nc.vector.tensor_tensor(pc, pc, ctxA[qb - 8][:], ADD)
            ctx_finish(pc, qb, div_act=True, add_dve=(qb % 4 >= 2))


        # ================= emission schedule ================================
        def units(fn, idxs):
            return [lambda i=i: fn(*i) if isinstance(i, tuple) else fn(i)
                    for i in idxs]

        def interleave(a, b, ratio):
            a = list(a)
            b = list(b)
            ia = ib = 0
            credit = 0.0
            while ia < len(a) or ib < len(b):
                if ia < len(a):
                    a[ia]()
                    ia += 1
                credit += ratio
                while credit >= 1.0 and ib < len(b):
                    b[ib]()
                    ib += 1
                    credit -= 1.0
            while ib < len(b):
                b[ib]()
                ib += 1

        # --- phase 1: QT nq0/nq1 chunk-paced as the first d1 chunks land
        #     (d1 loads first; no PE gap so the p-state stays hot), then
        #     KT nk0 the moment d2T lands.  Scores-path evicts split
        #     ACT (dc0) / DVE (dc1). ---
        for nq in range(2):
            for h in range(2):
                qt_half(0, nq, h)
                qt_half(1, nq, h)
            qt_evict(0, nq, sc_act=True, rs_act=False)
            qt_evict(1, nq, sc_act=False, rs_act=True)
        for dc in range(2):
            kt_unit(dc, 0, on_act=(dc == 0))
        for qg in range(2):
            for dc in range(2):
                qres_xbar(qg, dc)

        # --- phase 2: scores-h0 interleaved with QT nq2/3 + key transposes ---
        def qres_late(qg):
            for dc in range(2):
                qres_xbar(qg, dc)

        filler = (
            [lambda: kt_unit(0, 1, False), lambda: kt_unit(1, 1, False),
             lambda: kt_unit(0, 2, False), lambda: kt_unit(1, 2, False),
             lambda: qt_unit(0, 2, False, False),
             lambda: qt_unit(1, 2, False, False),
             lambda: kt_unit(0, 3, False), lambda: kt_unit(1, 3, False),
             lambda: qt_unit(0, 3, False, False),
             lambda: qt_unit(1, 3, False, False),
             lambda: qres_late(2), lambda: qres_late(3)]
            + units(key_tr, [0, 1, 2, 3, 4, 5, 6, 7])
        )
        scores_h0 = units(scores_unit, [(km, 0) for km in range(KB)])
        interleave(scores_h0, filler, len(filler) / len(scores_h0))

        # --- phase 3: scores-h1 with ctx-h0 and ctx-h1-A ---
        # ctx_h1_A reads exps km0-13 of h1, so A units may only be emitted
        # after scores-h1 km13 (Tile orders by emission-time dependencies)
        sc_h1_a = units(scores_unit, [(km, 1) for km in range(2 * KP_A)])
        sc_h1_b = units(scores_unit, [(km, 1) for km in range(2 * KP_A, KB)])
        ctx0 = units(ctx_unit_h0, list(range(0, 8)))
        ctxa = units(ctx_h1_A, list(range(8, 16)))
        for u in sc_h1_a[:5]:
            u()
        interleave(sc_h1_a[5:], ctx0, len(ctx0) / len(sc_h1_a[5:]))
        for u in sc_h1_b:
            u()
        for u in ctxa:
            u()

        # --- phase 4: ctx-h1-B tail ---
        for qb in range(8, 16):
            ctx_h1_B(qb)

    nc.compile()
    return nc


_NC = None
_last_in_maps = None


def make_host_inputs(data1_b, data2_b, Wq, bq, Wk, bk):
    """Pack one batch element's inputs into the device layout (bf16 + f32).

    d1t[n, p, j, q'] = data1[n*512 + q', 8p + j]   (q-chunked, i p-major)
    d2t[p, j, k]     = data2[k, 2p + j]
    wq[p, j*256+d]   = Wq[8p + j, d]; wk[p, j*256+d] = Wk[2p + j, d]
    """
    bf = ml_dtypes.bfloat16
    a1 = np.asarray(data1_b, np.float32).astype(bf)      # [2048, 1024]
    d1t = np.ascontiguousarray(
        a1.reshape(8, 256, 128, J1).transpose(0, 2, 3, 1))
    f8 = ml_dtypes.float8_e4m3
    a2 = np.asarray(data2_b, np.float32).astype(f8)      # [2048, 256]
    d2t = np.ascontiguousarray(
        a2.reshape(LK, 128, J2).transpose(1, 2, 0))
    Wq = np.asarray(Wq, dtype=np.float32)
    Wk = np.asarray(Wk, dtype=np.float32)
    bq = np.asarray(bq, dtype=np.float32)
    bk = np.asarray(bk, dtype=np.float32)
    wq = np.ascontiguousarray(
        Wq.astype(bf).reshape(128, J1, D).reshape(128, J1 * D))
    wk = np.ascontiguousarray(
        Wk.astype(f8).reshape(128, J2, D).reshape(128, J2 * D))
    bias = np.empty((128, 4), np.float32)
    bqk = bq + bk
    for c in range(2):
        bias[:, c] = bq[c * 128:(c + 1) * 128]
        bias[:, 2 + c] = bqk[c * 128:(c + 1) * 128]
    return {"d1t": d1t, "d2t": d2t, "wq": wq, "wk": wk, "bias": bias}


def _get_nc():
    global _NC
    if _NC is None:
        _NC = _build()
    return _NC


def kernel(data1, data2, Wq, bq, Wk, bk):
    global _last_in_maps
    data1 = np.asarray(data1, dtype=np.float32)
    data2 = np.asarray(data2, dtype=np.float32)

    nc = _get_nc()
    shared = None
    in_maps = []
    for b in range(B):
        m = make_host_inputs(data1[b], data2[b], Wq, bq, Wk, bk)
        if shared is None:
            shared = {k: m[k] for k in ("wq", "wk", "bias")}
        m.update(shared)
        in_maps.append(m)
    _last_in_maps = in_maps
    res = run_bass_kernel_spmd(nc, in_maps, core_ids=list(range(N_CORES)))
    return np.stack([res.results[i]["out"] for i in range(B)], axis=0)
